# revision 1
# baseline (speedup 1.0000x reference)
"""GroupedQueryAttention (B=1, S=4096, D=1024, G=16 heads, DH=64) on 8 TRN2 NeuronCores.

Sharding: tensor-parallel over heads. Core c computes heads {2c, 2c+1}:
  - Q/K/V projections with column-sliced weights (128 out-dims per core),
    producing Q^T/K^T/V^T in [dout, seq] layout (host pre-transposes inputs).
  - Flash-style attention without max-subtraction (scores are tiny:
    |s/8| < ~3), exp on ScalarE with fused 1/8 scale + per-key mask bias.
  - Softmax denominator comes free via an ones-column appended to V in the
    PV matmul (PSUM row 64 accumulates sum_k exp).
  - Output projection with row-sliced Wo produces a partial (4096, 1024)
    output per core; host sums the 8 partials and adds bo.

All matmuls run as float32r (full PE rate at ~fp32 precision).
"""

import os
import sys

for _p in ("/opt/trn_rl_repo", "/root/.axon_site/_ro/trn_rl_repo"):
    if os.path.isdir(_p) and _p not in sys.path:
        sys.path.insert(0, _p)

from contextlib import ExitStack

import numpy as np

import concourse.bass as bass
import concourse.mybir as mybir
import concourse.tile as tile
from concourse import bacc
from concourse.bass_utils import run_bass_kernel_spmd
from concourse.masks import make_identity

S = 4096          # sequence length
D = 1024          # model dim
G = 16            # heads
DH = 64           # head dim
P = 128           # partitions
QT = 512          # q-tile (moving free dim)
KC = 128          # k-chunk
NCORES = 8
HPC = G // NCORES             # heads per core = 2
N_ST = S // QT                # 8 s-tiles of 512
N_KCH = D // P                # 8 contraction chunks for projections
N_KC = S // KC                # 32 k-chunks for attention
DSL = P                       # per-core dout slice (2 heads * 64)

F32 = mybir.dt.float32
F32R = mybir.dt.float32r

_CACHE = {}


def _round_f32r(a):
    """Round fp32 -> fp32r (fp32 with the low 12 mantissa bits dropped)."""
    u = np.ascontiguousarray(a, dtype=np.float32).view(np.uint32)
    u = ((u + np.uint32(0x800)) & np.uint32(0xFFFFF000)).astype(np.uint32)
    return u.view(np.float32)


def _build_nc(dbg=False):
    key = ("nc", dbg)
    if key in _CACHE:
        return _CACHE[key]

    nc = bacc.Bacc(
        "TRN2", target_bir_lowering=False, debug=False, num_devices=NCORES
    )

    xqT = nc.dram_tensor("xqT", [D, S], F32R, kind="ExternalInput").ap()
    xkT = nc.dram_tensor("xkT", [D, S], F32R, kind="ExternalInput").ap()
    xvT = nc.dram_tensor("xvT", [D, S], F32R, kind="ExternalInput").ap()
    wqT = nc.dram_tensor("wqT", [N_KCH, P, DSL], F32R, kind="ExternalInput").ap()
    wkT = nc.dram_tensor("wkT", [N_KCH, P, DSL], F32R, kind="ExternalInput").ap()
    wvT = nc.dram_tensor("wvT", [N_KCH, P, DSL], F32R, kind="ExternalInput").ap()
    woT = nc.dram_tensor("woT", [DSL, D], F32R, kind="ExternalInput").ap()
    bq = nc.dram_tensor("bq", [DSL, 1], F32, kind="ExternalInput").ap()
    bk = nc.dram_tensor("bk", [DSL, 1], F32, kind="ExternalInput").ap()
    bv = nc.dram_tensor("bv", [DSL, 1], F32, kind="ExternalInput").ap()
    mbias = nc.dram_tensor("mbias", [P, N_KC], F32, kind="ExternalInput").ap()
    out_d = nc.dram_tensor("out", [S, D], F32, kind="ExternalOutput").ap()
    if dbg:
        dbg_d = {
            n: nc.dram_tensor(f"dbg_{n}", shp, F32, kind="ExternalOutput").ap()
            for n, shp in (
                ("qts", [P, S]), ("kts", [P, S]), ("vts", [P, S]),
                ("vn0", [P, N_KC * (DH + 4)]), ("vn1", [P, N_KC * (DH + 4)]),
                ("attnT", [P, S]),
            )
        }

    with tile.TileContext(nc) as tc, ExitStack() as ctx:
        consts = ctx.enter_context(tc.tile_pool(name="consts", bufs=1))
        big = ctx.enter_context(tc.tile_pool(name="big", bufs=1))
        xin = ctx.enter_context(tc.tile_pool(name="xin", bufs=6))
        et_pool = ctx.enter_context(tc.tile_pool(name="et", bufs=4))
        small = ctx.enter_context(tc.tile_pool(name="small", bufs=4))
        oevict = ctx.enter_context(tc.tile_pool(name="oevict", bufs=4))
        ps_mm = ctx.enter_context(tc.tile_pool(name="ps_mm", bufs=4, space="PSUM"))
        ps_pv = ctx.enter_context(tc.tile_pool(name="ps_pv", bufs=2, space="PSUM"))
        ps_tr = ctx.enter_context(tc.tile_pool(name="ps_tr", bufs=2, space="PSUM"))

        # ---- constants ----
        ident = consts.tile([P, P], F32)
        make_identity(nc, ident[:])

        w_s = {}
        for name, wd in (("q", wqT), ("k", wkT), ("v", wvT)):
            w = consts.tile([P, N_KCH * DSL], F32R, tag=f"w{name}")
            for kc in range(N_KCH):
                nc.sync.dma_start(w[:, kc * DSL:(kc + 1) * DSL], wd[kc])
            w_s[name] = w
        wo_s = consts.tile([DSL, D], F32R, tag="wo")
        nc.sync.dma_start(wo_s[:], woT)
        b_s = {}
        for name, bd in (("q", bq), ("k", bk), ("v", bv)):
            b = consts.tile([DSL, 1], F32, tag=f"b{name}")
            nc.sync.dma_start(b[:], bd)
            b_s[name] = b
        mb_s = consts.tile([P, N_KC], F32, tag="mb")
        nc.sync.dma_start(mb_s[:], mbias)

        # ---- resident activations ----
        QTs = big.tile([P, S], F32R, tag="QTs")      # Q^T  [dout, s]
        KTs = big.tile([P, S], F32R, tag="KTs")      # K^T  [dout, s]
        VTs = big.tile([P, S], F32, tag="VTs")      # V^T  [dout, s]
        # V natural per head: [k-part, chunk, DH+1]; col DH is the ones
        # column that makes PV also accumulate sum_k exp (softmax denom).
        Vnat = [
            big.tile([P, N_KC, DH + 4], F32R, tag=f"Vn{h}", name=f"Vnat{h}")
            for h in range(HPC)
        ]
        attnT = big.tile([P, S], F32R, tag="attnT")  # normalized attn^T [din, s]

        ones_col = consts.tile([P, 1], F32, tag="ones")
        nc.vector.memset(ones_col[:], 1.0)
        for h in range(HPC):
            nc.vector.tensor_copy(
                Vnat[h][:, :, DH:DH + 1],
                ones_col[:].to_broadcast((P, N_KC, 1)),
            )

        # ---- phase 1: projections (streamed over s-tiles) ----
        for st in range(N_ST):
            sl = slice(st * QT, (st + 1) * QT)
            for name, xd, dst in (("k", xkT, KTs), ("q", xqT, QTs), ("v", xvT, VTs)):
                ps = ps_mm.tile([P, QT], F32, tag="mm")
                for kc in range(N_KCH):
                    xt = xin.tile([P, QT], F32R, tag="xt")
                    nc.sync.dma_start(xt[:], xd[kc * P:(kc + 1) * P, sl])
                    nc.tensor.matmul(
                        ps[:],
                        w_s[name][:, kc * DSL:(kc + 1) * DSL],
                        xt[:],
                        start=(kc == 0),
                        stop=(kc == N_KCH - 1),
                    )
                nc.scalar.activation(
                    dst[:, sl], ps[:],
                    mybir.ActivationFunctionType.Identity,
                    bias=b_s[name][:], scale=1.0,
                )
            # transpose this s-tile of V^T into V natural (4 k-chunks)
            for h in range(HPC):
                hs = slice(h * DH, (h + 1) * DH)
                for j in range(QT // KC):
                    kc = st * (QT // KC) + j
                    pt = ps_tr.tile([P, DH], F32, tag="tr")
                    nc.tensor.transpose(
                        pt[:], VTs[hs, kc * KC:(kc + 1) * KC], ident[hs, hs]
                    )
                    nc.vector.tensor_copy(Vnat[h][:, kc, 0:DH], pt[:])

        # ---- phase 2+3: attention + output projection ----
        for qt in range(N_ST):
            qsl = slice(qt * QT, (qt + 1) * QT)
            for h in range(HPC):
                hs = slice(h * DH, (h + 1) * DH)
                pv = ps_pv.tile([DH + 1, QT], F32, tag="pv")
                for kc in range(N_KC):
                    sc = ps_mm.tile([P, QT], F32, tag="mm")
                    nc.tensor.matmul(
                        sc[:],
                        KTs[hs, kc * KC:(kc + 1) * KC],
                        QTs[hs, qsl],
                        start=True, stop=True,
                    )
                    et = et_pool.tile([P, QT], F32R, tag="et")
                    nc.scalar.activation(
                        et[:], sc[:],
                        mybir.ActivationFunctionType.Exp,
                        bias=mb_s[:, kc:kc + 1], scale=0.125,
                    )
                    nc.tensor.matmul(
                        pv[:],
                        Vnat[h][:, kc, 0:DH + 1],
                        et[:],
                        start=(kc == 0), stop=(kc == N_KC - 1),
                    )
                # normalize: attnT[h, qsl] = pv[0:DH] * (1/pv[DH])
                rec = small.tile([1, QT], F32, tag="rec")
                nc.vector.reciprocal(rec[:], pv[DH:DH + 1, :])
                bc = small.tile([DH, QT], F32, tag="bc")
                nc.gpsimd.partition_broadcast(bc[:], rec[:])
                nc.vector.tensor_mul(attnT[hs, qsl], pv[0:DH, :], bc[:])
            # output projection for the 4 s-tiles of this q-tile
            for j in range(QT // P):
                st = qt * (QT // P) + j
                for nt in range(D // QT):
                    po = ps_mm.tile([P, QT], F32, tag="mm")
                    nc.tensor.matmul(
                        po[:],
                        attnT[:, st * P:(st + 1) * P],
                        wo_s[:, nt * QT:(nt + 1) * QT],
                        start=True, stop=True,
                    )
                    ot = oevict.tile([P, QT], F32, tag="ot")
                    nc.vector.tensor_copy(ot[:], po[:])
                    nc.sync.dma_start(
                        out_d[st * P:(st + 1) * P, nt * QT:(nt + 1) * QT], ot[:]
                    )

        if dbg:
            for name, t in (("qts", QTs), ("kts", KTs), ("vts", VTs),
                            ("attnT", attnT)):
                nc.sync.dma_start(dbg_d[name][:, :], t[:].bitcast(F32))
            nc.sync.dma_start(dbg_d["vn0"][:, :], Vnat[0][:].bitcast(F32))
            nc.sync.dma_start(dbg_d["vn1"][:, :], Vnat[1][:].bitcast(F32))

    nc.compile()
    _CACHE[key] = nc
    return nc


def _prep_in_maps(query, key, value, mask, Wq, bq, Wk, bk, Wv, bv, Wo, bo):
    f = np.float32
    qT = _round_f32r(np.asarray(query)[0].T)
    kT = _round_f32r(np.asarray(key)[0].T)
    vT = _round_f32r(np.asarray(value)[0].T)
    mb = np.where(np.asarray(mask)[0] == 0, f(-1e9), f(0.0)).astype(f)
    mb = np.ascontiguousarray(mb.reshape(N_KC, KC).T)  # [128, 32]
    WqT, WkT, WvT, WoT = (_round_f32r(np.asarray(W).T)
                          for W in (Wq, Wk, Wv, Wo))
    in_maps = []
    for c in range(NCORES):
        cs = slice(c * DSL, (c + 1) * DSL)
        in_maps.append({
            "xqT": qT, "xkT": kT, "xvT": vT,
            "wqT": np.ascontiguousarray(WqT[:, cs]).reshape(N_KCH, P, DSL),
            "wkT": np.ascontiguousarray(WkT[:, cs]).reshape(N_KCH, P, DSL),
            "wvT": np.ascontiguousarray(WvT[:, cs]).reshape(N_KCH, P, DSL),
            "woT": np.ascontiguousarray(WoT[cs, :]),
            "bq": np.ascontiguousarray(bq[cs].astype(f, copy=False)).reshape(DSL, 1),
            "bk": np.ascontiguousarray(bk[cs].astype(f, copy=False)).reshape(DSL, 1),
            "bv": np.ascontiguousarray(bv[cs].astype(f, copy=False)).reshape(DSL, 1),
            "mbias": mb,
        })
    return in_maps


def run(inputs, trace=False, trace_kwargs=None, dbg=False):
    nc = _build_nc(dbg=dbg)
    in_maps = _prep_in_maps(**inputs)
    res = run_bass_kernel_spmd(
        nc, in_maps, core_ids=list(range(NCORES)), trace=trace,
        **(trace_kwargs or {}),
    )
    bo = np.asarray(inputs["bo"], dtype=np.float32)
    acc = np.zeros((S, D), dtype=np.float32)
    for r in res.results:
        acc += r["out"]
    out = (acc + bo[None, :]).astype(np.float32)[None]
    return out, res


def kernel(**inputs):
    out, _ = run(inputs, trace=False)
    return out



# revision 21
# speedup vs baseline: 1.6720x; 1.6720x over previous
"""GroupedQueryAttention (B=1, S=4096, D=1024, G=16 heads, DH=64) on 8 TRN2 NeuronCores.

Sharding: tensor-parallel over heads. Core c computes heads {2c, 2c+1}:
  - Q/K/V projections with column-sliced weights (128 out-dims per core),
    producing Q^T/K^T/V^T in [dout, seq] layout (host pre-transposes inputs).
    All attention-path matmuls run in bf16 (fp32 PSUM accumulation); bias
    adds are done on VectorE (tensor_scalar_add) to keep ScalarE free for exp.
  - Flash-style attention without max-subtraction (scores are tiny:
    |s/8| < ~3). The two heads' QK^T matmuls are row-packed (K=64 each,
    tile_position rows 0-63 / 64-127) so they run concurrently on the PE.
  - exp on ScalarE over two PSUM banks at once (N=1024) with fused 1/8
    scale. The attention mask is folded multiplicatively into V-natural
    (V rows and the appended ones-column are zeroed for masked keys), so
    exp needs no per-chunk bias and softmax stays exact for any 0/1 mask.
  - Softmax denominator comes free via the ones-column appended to V in the
    PV matmul (PSUM row 64 accumulates sum_k exp).
  - Output projection with row-sliced Wo produces a partial (4096, 1024)
    output per core; host sums the 8 partials and adds bo.
"""

import os
import sys

for _p in ("/opt/trn_rl_repo", "/root/.axon_site/_ro/trn_rl_repo"):
    if os.path.isdir(_p) and _p not in sys.path:
        sys.path.insert(0, _p)

from contextlib import ExitStack

import numpy as np
import ml_dtypes

import concourse.bass as bass
import concourse.mybir as mybir
import concourse.tile as tile
from concourse import bacc
from concourse.bass_utils import run_bass_kernel_spmd
from concourse.masks import make_identity

S = 4096          # sequence length
D = 1024          # model dim
G = 16            # heads
DH = 64           # head dim
P = 128           # partitions
QT = 512          # q-tile (moving free dim)
KC = 128          # k-chunk
NCORES = 8
HPC = G // NCORES             # heads per core = 2
N_ST = S // QT                # 8 s-tiles of 512
N_KCH = D // P                # 8 contraction chunks for projections
N_KC = S // KC                # 32 k-chunks for attention
DSL = P                       # per-core dout slice (2 heads * 64)

F32 = mybir.dt.float32
BF16 = mybir.dt.bfloat16
BF16_NP = ml_dtypes.bfloat16

_CACHE = {}


def _build_nc(dbg=False):
    key = ("nc", dbg)
    if key in _CACHE:
        return _CACHE[key]

    nc = bacc.Bacc(
        "TRN2", target_bir_lowering=False, debug=False, num_devices=NCORES
    )

    xqT = nc.dram_tensor("xqT", [D, S], BF16, kind="ExternalInput").ap()
    xkT = nc.dram_tensor("xkT", [D, S], BF16, kind="ExternalInput").ap()
    xvT = nc.dram_tensor("xvT", [D, S], BF16, kind="ExternalInput").ap()
    wqT = nc.dram_tensor("wqT", [N_KCH, P, DSL], BF16, kind="ExternalInput").ap()
    wkT = nc.dram_tensor("wkT", [N_KCH, P, DSL], BF16, kind="ExternalInput").ap()
    wvT = nc.dram_tensor("wvT", [N_KCH, P, DSL], BF16, kind="ExternalInput").ap()
    woT = nc.dram_tensor("woT", [DSL, D], BF16, kind="ExternalInput").ap()
    bq = nc.dram_tensor("bq", [DSL, 1], F32, kind="ExternalInput").ap()
    bk = nc.dram_tensor("bk", [DSL, 1], F32, kind="ExternalInput").ap()
    bv = nc.dram_tensor("bv", [DSL, 1], F32, kind="ExternalInput").ap()
    # multiplicative 0/1 mask, [key-in-chunk, chunk]
    mmul = nc.dram_tensor("mmul", [P, N_KC], F32, kind="ExternalInput").ap()
    out_d = nc.dram_tensor("out", [S, D], F32, kind="ExternalOutput").ap()
    if dbg:
        dbg_d = {
            n: nc.dram_tensor(f"dbg_{n}", shp, dt, kind="ExternalOutput").ap()
            for n, shp, dt in (
                ("qts", [P, S], BF16), ("kts", [P, S], BF16),
                ("vts", [P, S], F32),
                ("vn0", [P, N_KC * (DH + 4)], BF16),
                ("vn1", [P, N_KC * (DH + 4)], BF16),
                ("attnT", [P, S], BF16),
                ("et0", [P, 2 * QT], BF16),
                ("den0", [DH + 1, QT], F32),
            )
        }

    with tile.TileContext(nc) as tc, ExitStack() as ctx:
        consts = ctx.enter_context(tc.tile_pool(name="consts", bufs=1))
        big = ctx.enter_context(tc.tile_pool(name="big", bufs=1))
        xin = ctx.enter_context(tc.tile_pool(name="xin", bufs=6))
        et_pool = ctx.enter_context(tc.tile_pool(name="et", bufs=4))
        small = ctx.enter_context(tc.tile_pool(name="small", bufs=4))
        oevict = ctx.enter_context(tc.tile_pool(name="oevict", bufs=4))
        ps_qk = ctx.enter_context(tc.tile_pool(name="ps_qk", bufs=2, space="PSUM"))
        ps_pv = ctx.enter_context(tc.tile_pool(name="ps_pv", bufs=2, space="PSUM"))
        ps_o = ctx.enter_context(tc.tile_pool(name="ps_o", bufs=2, space="PSUM"))

        # ---- constants ----
        ident = consts.tile([P, P], F32)
        make_identity(nc, ident[:])

        w_s = {}
        for name, wd in (("q", wqT), ("k", wkT), ("v", wvT)):
            w = consts.tile([P, N_KCH * DSL], BF16, tag=f"w{name}")
            for kc in range(N_KCH):
                nc.sync.dma_start(w[:, kc * DSL:(kc + 1) * DSL], wd[kc])
            w_s[name] = w
        wo_s = consts.tile([DSL, D], BF16, tag="wo")
        nc.sync.dma_start(wo_s[:], woT)
        b_s = {}
        for name, bd in (("q", bq), ("k", bk), ("v", bv)):
            b = consts.tile([DSL, 1], F32, tag=f"b{name}")
            nc.sync.dma_start(b[:], bd)
            b_s[name] = b
        mm_s = consts.tile([P, N_KC], F32, tag="mm")
        nc.sync.dma_start(mm_s[:], mmul)

        # ---- resident activations ----
        QTs = big.tile([P, S], BF16, tag="QTs")      # Q^T  [dout, s]
        KTs = big.tile([P, S], BF16, tag="KTs")      # K^T  [dout, s]
        VTs = big.tile([P, S], F32, tag="VTs")       # V^T  [dout, s]
        # V natural per head: [k-part, chunk, DH+1]; col DH is the mask
        # column (1/0) that makes PV also accumulate sum_k exp (softmax
        # denom) restricted to unmasked keys.
        Vnat = [
            big.tile([P, N_KC, DH + 4], BF16, tag=f"Vn{h}", name=f"Vnat{h}")
            for h in range(HPC)
        ]
        attnT = big.tile([P, S], BF16, tag="attnT")  # normalized attn^T [din, s]

        for h in range(HPC):
            # mask column: exactly the 0/1 mask per key
            nc.vector.tensor_copy(Vnat[h][:, :, DH], mm_s[:])

        # ---- phase 1: projections (streamed over s-tiles) ----
        # emit K, V, Q per s-tile so attention q-tile 0 can start early
        for st in range(N_ST):
            sl = slice(st * QT, (st + 1) * QT)
            for name, xd in (("k", xkT), ("v", xvT), ("q", xqT)):
                ps = ps_o.tile([P, QT], F32, tag="mm", name="ps")
                for kc in range(N_KCH):
                    xt = xin.tile([P, QT], BF16, tag="xt")
                    nc.sync.dma_start(xt[:], xd[kc * P:(kc + 1) * P, sl])
                    nc.tensor.matmul(
                        ps[:],
                        w_s[name][:, kc * DSL:(kc + 1) * DSL],
                        xt[:],
                        start=(kc == 0),
                        stop=(kc == N_KCH - 1),
                    )
                # bias add + cast on VectorE (keeps ScalarE free for exp)
                if name == "q":
                    nc.vector.tensor_scalar_add(QTs[:, sl], ps[:], b_s["q"][:])
                elif name == "k":
                    nc.vector.tensor_scalar_add(KTs[:, sl], ps[:], b_s["k"][:])
                else:
                    nc.vector.tensor_scalar_add(VTs[:, sl], ps[:], b_s["v"][:])
            # transpose this s-tile of V^T into V natural (4 k-chunks),
            # applying the multiplicative key mask
            for h in range(HPC):
                hs = slice(h * DH, (h + 1) * DH)
                for j in range(QT // KC):
                    kc = st * (QT // KC) + j
                    pt = ps_qk.tile([P, DH], F32, tag="qk", name="pt")
                    nc.tensor.transpose(
                        pt[:], VTs[hs, kc * KC:(kc + 1) * KC], ident[hs, hs]
                    )
                    nc.vector.tensor_mul(
                        Vnat[h][:, kc, 0:DH],
                        pt[:],
                        mm_s[:, kc:kc + 1].to_broadcast((P, DH)),
                    )

        # ---- phase 2+3: attention + output projection ----
        for qt in range(N_ST):
            qsl = slice(qt * QT, (qt + 1) * QT)
            pv = [
                ps_pv.tile([DH + 1, QT], F32, tag="pv", name=f"pv{h}")
                for h in range(HPC)
            ]
            for kcp in range(N_KC // 2):
                kc0, kc1 = 2 * kcp, 2 * kcp + 1
                # two heads' QK^T row-packed on the PE (rows 0-63 / 64-127)
                sc = [
                    ps_qk.tile([P, 2 * QT], F32, tag="qk", name=f"sc{h}")
                    for h in range(HPC)
                ]
                for j, kc in ((0, kc0), (1, kc1)):
                    for h in range(HPC):
                        hs = slice(h * DH, (h + 1) * DH)
                        nc.tensor.matmul(
                            sc[h][:, j * QT:(j + 1) * QT],
                            KTs[hs, kc * KC:(kc + 1) * KC],
                            QTs[hs, qsl],
                            start=True, stop=True,
                        )
                for h in range(HPC):
                    et = et_pool.tile([P, 2 * QT], BF16, tag="et")
                    nc.scalar.activation(
                        et[:], sc[h][:],
                        mybir.ActivationFunctionType.Exp,
                        scale=0.125,
                    )
                    if dbg and qt == 0 and kcp == 0 and h == 0:
                        nc.sync.dma_start(dbg_d["et0"][:, :], et[:])
                    for j, kc in ((0, kc0), (1, kc1)):
                        nc.tensor.matmul(
                            pv[h][:],
                            Vnat[h][:, kc, 0:DH + 1],
                            et[:, j * QT:(j + 1) * QT],
                            start=(kc == 0), stop=(kc == N_KC - 1),
                        )
            if dbg and qt == 0:
                pvd = small.tile([DH + 1, QT], F32, tag="pvd", name="pvd")
                nc.vector.tensor_copy(pvd[:], pv[0][:])
                nc.sync.dma_start(dbg_d["den0"][:, :], pvd[:])
            # normalize: attnT[h, qsl] = pv[0:DH] * (1/pv[DH])
            for h in range(HPC):
                hs = slice(h * DH, (h + 1) * DH)
                rec = small.tile([1, QT], F32, tag="rec")
                nc.vector.reciprocal(rec[:], pv[h][DH:DH + 1, :])
                bc = small.tile([DH, QT], F32, tag="bc")
                nc.gpsimd.partition_broadcast(bc[:], rec[:])
                nc.vector.tensor_mul(attnT[hs, qsl], pv[h][0:DH, :], bc[:])
            # output projection for the 4 s-tiles of this q-tile
            for j in range(QT // P):
                st = qt * (QT // P) + j
                for nt in range(D // QT):
                    po = ps_o.tile([P, QT], F32, tag="mm", name="po")
                    nc.tensor.matmul(
                        po[:],
                        attnT[:, st * P:(st + 1) * P],
                        wo_s[:, nt * QT:(nt + 1) * QT],
                        start=True, stop=True,
                    )
                    ot = oevict.tile([P, QT], F32, tag="ot")
                    nc.vector.tensor_copy(ot[:], po[:])
                    nc.sync.dma_start(
                        out_d[st * P:(st + 1) * P, nt * QT:(nt + 1) * QT], ot[:]
                    )

        if dbg:
            for name, t in (("qts", QTs), ("kts", KTs), ("vts", VTs),
                            ("attnT", attnT)):
                nc.sync.dma_start(dbg_d[name][:, :], t[:])
            nc.sync.dma_start(dbg_d["vn0"][:, :], Vnat[0][:])
            nc.sync.dma_start(dbg_d["vn1"][:, :], Vnat[1][:])

    nc.compile()
    _CACHE[key] = nc
    return nc


def _prep_in_maps(query, key, value, mask, Wq, bq, Wk, bk, Wv, bv, Wo, bo):
    f = np.float32
    qT = np.asarray(query, dtype=f)[0].T.astype(BF16_NP)
    kT = np.asarray(key, dtype=f)[0].T.astype(BF16_NP)
    vT = np.asarray(value, dtype=f)[0].T.astype(BF16_NP)
    mm = (np.asarray(mask)[0] != 0).astype(f)
    mm = np.ascontiguousarray(mm.reshape(N_KC, KC).T)  # [128, 32]
    WqT, WkT, WvT, WoT = (np.asarray(W, dtype=f).T.astype(BF16_NP)
                          for W in (Wq, Wk, Wv, Wo))
    in_maps = []
    for c in range(NCORES):
        cs = slice(c * DSL, (c + 1) * DSL)
        in_maps.append({
            "xqT": qT, "xkT": kT, "xvT": vT,
            "wqT": np.ascontiguousarray(WqT[:, cs]).reshape(N_KCH, P, DSL),
            "wkT": np.ascontiguousarray(WkT[:, cs]).reshape(N_KCH, P, DSL),
            "wvT": np.ascontiguousarray(WvT[:, cs]).reshape(N_KCH, P, DSL),
            "woT": np.ascontiguousarray(WoT[cs, :]),
            "bq": np.ascontiguousarray(bq[cs].astype(f, copy=False)).reshape(DSL, 1),
            "bk": np.ascontiguousarray(bk[cs].astype(f, copy=False)).reshape(DSL, 1),
            "bv": np.ascontiguousarray(bv[cs].astype(f, copy=False)).reshape(DSL, 1),
            "mmul": mm,
        })
    return in_maps


def run(inputs, trace=False, trace_kwargs=None, dbg=False):
    nc = _build_nc(dbg=dbg)
    in_maps = _prep_in_maps(**inputs)
    res = run_bass_kernel_spmd(
        nc, in_maps, core_ids=list(range(NCORES)), trace=trace,
        **(trace_kwargs or {}),
    )
    bo = np.asarray(inputs["bo"], dtype=np.float32)
    acc = np.zeros((S, D), dtype=np.float32)
    for r in res.results:
        acc += r["out"]
    out = (acc + bo[None, :]).astype(np.float32)[None]
    return out, res


def kernel(**inputs):
    out, _ = run(inputs, trace=False)
    return out


# revision 22
# speedup vs baseline: 1.8607x; 1.1129x over previous
"""GroupedQueryAttention (B=1, S=4096, D=1024, G=16 heads, DH=64) on 8 TRN2 NeuronCores.

Sharding: tensor-parallel over heads. Core c computes heads {2c, 2c+1}:
  - Q/K/V projections with column-sliced weights (128 out-dims per core),
    producing Q^T/K^T/V^T in [dout, seq] layout (host pre-transposes inputs).
    Inputs stream in as 512 KB half-row chunks on two HWDGE queues (sync for
    K/V, scalar for Q) so the load runs near HBM line rate. All attention-path
    matmuls are bf16 (fp32 PSUM accumulation); bias adds run on VectorE.
  - Flash-style attention without max-subtraction (scores are tiny:
    |s/8| < ~3). The two heads' QK^T matmuls are row-packed (K=64 each,
    tile_position rows 0-63 / 64-127) so they run concurrently on the PE.
  - exp on ScalarE over two PSUM banks at once (N=1024) with fused 1/8
    scale. The attention mask is folded multiplicatively into V-natural
    (V rows and the appended ones-column are zeroed for masked keys), so
    exp needs no per-chunk bias and softmax stays exact for any 0/1 mask.
  - Softmax denominator comes free via the ones-column appended to V in the
    PV matmul (PSUM row 64 accumulates sum_k exp).
  - Output projection with row-sliced Wo produces a partial (4096, 1024)
    output per core; each q-tile's output projection is deferred into the
    next q-tile's QK/PV stream so it fills PE slack instead of stalling
    ScalarE at tile boundaries. Host sums the 8 partials and adds bo.
"""

import os
import sys

for _p in ("/opt/trn_rl_repo", "/root/.axon_site/_ro/trn_rl_repo"):
    if os.path.isdir(_p) and _p not in sys.path:
        sys.path.insert(0, _p)

from contextlib import ExitStack

import numpy as np
import ml_dtypes

import concourse.bass as bass
import concourse.mybir as mybir
import concourse.tile as tile
from concourse import bacc
from concourse.bass_utils import run_bass_kernel_spmd
from concourse.masks import make_identity

S = 4096          # sequence length
D = 1024          # model dim
G = 16            # heads
DH = 64           # head dim
P = 128           # partitions
QT = 512          # q-tile (moving free dim)
KC = 128          # k-chunk
NCORES = 8
HPC = G // NCORES             # heads per core = 2
N_ST = S // QT                # 8 s-tiles of 512
N_KCH = D // P                # 8 contraction chunks for projections
N_KC = S // KC                # 32 k-chunks for attention
DSL = P                       # per-core dout slice (2 heads * 64)
XH = S // 2                   # input DMA chunk width (half row)

F32 = mybir.dt.float32
BF16 = mybir.dt.bfloat16
BF16_NP = ml_dtypes.bfloat16

_CACHE = {}


def _build_nc(dbg=False):
    key = ("nc", dbg)
    if key in _CACHE:
        return _CACHE[key]

    nc = bacc.Bacc(
        "TRN2", target_bir_lowering=False, debug=False, num_devices=NCORES
    )

    xqT = nc.dram_tensor("xqT", [D, S], BF16, kind="ExternalInput").ap()
    xkT = nc.dram_tensor("xkT", [D, S], BF16, kind="ExternalInput").ap()
    xvT = nc.dram_tensor("xvT", [D, S], BF16, kind="ExternalInput").ap()
    wqT = nc.dram_tensor("wqT", [N_KCH, P, DSL], BF16, kind="ExternalInput").ap()
    wkT = nc.dram_tensor("wkT", [N_KCH, P, DSL], BF16, kind="ExternalInput").ap()
    wvT = nc.dram_tensor("wvT", [N_KCH, P, DSL], BF16, kind="ExternalInput").ap()
    woT = nc.dram_tensor("woT", [DSL, D], BF16, kind="ExternalInput").ap()
    bq = nc.dram_tensor("bq", [DSL, 1], F32, kind="ExternalInput").ap()
    bk = nc.dram_tensor("bk", [DSL, 1], F32, kind="ExternalInput").ap()
    bv = nc.dram_tensor("bv", [DSL, 1], F32, kind="ExternalInput").ap()
    # multiplicative 0/1 mask, [key-in-chunk, chunk]
    mmul = nc.dram_tensor("mmul", [P, N_KC], F32, kind="ExternalInput").ap()
    out_d = nc.dram_tensor("out", [S, D], F32, kind="ExternalOutput").ap()
    if dbg:
        dbg_d = {
            n: nc.dram_tensor(f"dbg_{n}", shp, dt, kind="ExternalOutput").ap()
            for n, shp, dt in (
                ("qts", [P, S], BF16), ("kts", [P, S], BF16),
                ("vts", [P, S], F32),
                ("vn0", [P, N_KC * (DH + 4)], BF16),
                ("vn1", [P, N_KC * (DH + 4)], BF16),
                ("attnT", [P, S], BF16),
            )
        }

    with tile.TileContext(nc) as tc, ExitStack() as ctx:
        consts = ctx.enter_context(tc.tile_pool(name="consts", bufs=1))
        big = ctx.enter_context(tc.tile_pool(name="big", bufs=1))
        xkv_p = ctx.enter_context(tc.tile_pool(name="xkv", bufs=8))
        xq_p = ctx.enter_context(tc.tile_pool(name="xq", bufs=8))
        et_pool = ctx.enter_context(tc.tile_pool(name="et", bufs=10))
        small = ctx.enter_context(tc.tile_pool(name="small", bufs=4))
        oevict = ctx.enter_context(tc.tile_pool(name="oevict", bufs=4))
        ps_qk = ctx.enter_context(tc.tile_pool(name="ps_qk", bufs=2, space="PSUM"))
        ps_pv = ctx.enter_context(tc.tile_pool(name="ps_pv", bufs=2, space="PSUM"))
        ps_o = ctx.enter_context(tc.tile_pool(name="ps_o", bufs=2, space="PSUM"))

        # ---- constants ----
        ident = consts.tile([P, P], F32)
        make_identity(nc, ident[:])

        w_s = {}
        for name, wd in (("k", wkT), ("q", wqT), ("v", wvT)):
            w = consts.tile([P, N_KCH * DSL], BF16, tag=f"w{name}")
            for kc in range(N_KCH):
                nc.sync.dma_start(w[:, kc * DSL:(kc + 1) * DSL], wd[kc])
            w_s[name] = w
        wo_s = consts.tile([DSL, D], BF16, tag="wo")
        nc.sync.dma_start(wo_s[:], woT)
        b_s = {}
        for name, bd in (("q", bq), ("k", bk), ("v", bv)):
            b = consts.tile([DSL, 1], F32, tag=f"b{name}")
            nc.sync.dma_start(b[:], bd)
            b_s[name] = b
        mm_s = consts.tile([P, N_KC], F32, tag="mm")
        nc.sync.dma_start(mm_s[:], mmul)

        # ---- resident activations ----
        QTs = big.tile([P, S], BF16, tag="QTs")      # Q^T  [dout, s]
        KTs = big.tile([P, S], BF16, tag="KTs")      # K^T  [dout, s]
        VTs = big.tile([P, S], F32, tag="VTs")       # V^T  [dout, s]
        # V natural per head: [k-part, chunk, DH+1]; col DH is the mask
        # column (1/0) that makes PV also accumulate sum_k exp (softmax
        # denom) restricted to unmasked keys.
        Vnat = [
            big.tile([P, N_KC, DH + 4], BF16, tag=f"Vn{h}", name=f"Vnat{h}")
            for h in range(HPC)
        ]
        attnT = big.tile([P, S], BF16, tag="attnT")  # normalized attn^T [din, s]

        for h in range(HPC):
            # mask column: exactly the 0/1 mask per key
            nc.vector.tensor_copy(Vnat[h][:, :, DH], mm_s[:])

        # ---- phase 1: projections ----
        # K and V stream on the sync HWDGE queue, Q concurrently on the
        # scalar queue; 512 KB half-row chunks hit near HBM line rate.
        def proj_halves(name, xd, dma_eng, evict):
            for half in range(2):
                hsl = slice(half * XH, (half + 1) * XH)
                pool = xq_p if name == "q" else xkv_p
                xt = []
                for kc in range(N_KCH):
                    x = pool.tile([P, XH], BF16, tag="xf", name=f"x{name}{kc}")
                    dma_eng.dma_start(x[:], xd[kc * P:(kc + 1) * P, hsl])
                    xt.append(x)
                for j in range(XH // QT):
                    st = half * (XH // QT) + j
                    sl = slice(st * QT, (st + 1) * QT)
                    jsl = slice(j * QT, (j + 1) * QT)
                    ps = ps_o.tile([P, QT], F32, tag="mm", name="ps")
                    for kc in range(N_KCH):
                        nc.tensor.matmul(
                            ps[:],
                            w_s[name][:, kc * DSL:(kc + 1) * DSL],
                            xt[kc][:, jsl],
                            start=(kc == 0),
                            stop=(kc == N_KCH - 1),
                        )
                    evict(ps, sl, st)

        def evict_k(ps, sl, st):
            nc.vector.tensor_scalar_add(KTs[:, sl], ps[:], b_s["k"][:])

        def evict_q(ps, sl, st):
            nc.vector.tensor_scalar_add(QTs[:, sl], ps[:], b_s["q"][:])

        def evict_v(ps, sl, st):
            nc.vector.tensor_scalar_add(VTs[:, sl], ps[:], b_s["v"][:])
            # transpose this s-tile of V^T into V natural (4 k-chunks),
            # applying the multiplicative key mask
            for h in range(HPC):
                hs = slice(h * DH, (h + 1) * DH)
                for jj in range(QT // KC):
                    kc = st * (QT // KC) + jj
                    pt = ps_qk.tile([P, DH], F32, tag="qk", name="pt")
                    nc.tensor.transpose(
                        pt[:], VTs[hs, kc * KC:(kc + 1) * KC], ident[hs, hs]
                    )
                    nc.vector.tensor_mul(
                        Vnat[h][:, kc, 0:DH],
                        pt[:],
                        mm_s[:, kc:kc + 1].to_broadcast((P, DH)),
                    )

        proj_halves("k", xkT, nc.sync, evict_k)
        proj_halves("q", xqT, nc.scalar, evict_q)
        proj_halves("v", xvT, nc.sync, evict_v)

        # ---- phase 2+3: attention, with the previous q-tile's output
        # projection interleaved into the step stream ----
        def outproj_block(qt, blk):
            st = qt * (QT // P) + blk // 2
            nt = blk % 2
            po = ps_o.tile([P, QT], F32, tag="mm", name="po")
            nc.tensor.matmul(
                po[:],
                attnT[:, st * P:(st + 1) * P],
                wo_s[:, nt * QT:(nt + 1) * QT],
                start=True, stop=True,
            )
            ot = oevict.tile([P, QT], F32, tag="ot")
            nc.vector.tensor_copy(ot[:], po[:])
            nc.sync.dma_start(
                out_d[st * P:(st + 1) * P, nt * QT:(nt + 1) * QT], ot[:]
            )

        for qt in range(N_ST):
            qsl = slice(qt * QT, (qt + 1) * QT)
            pv = [
                ps_pv.tile([DH + 1, QT], F32, tag="pv", name=f"pv{h}")
                for h in range(HPC)
            ]
            for kcp in range(N_KC // 2):
                kc0, kc1 = 2 * kcp, 2 * kcp + 1
                # two heads' QK^T row-packed on the PE (rows 0-63 / 64-127)
                sc = [
                    ps_qk.tile([P, 2 * QT], F32, tag="qk", name=f"sc{h}")
                    for h in range(HPC)
                ]
                for j, kc in ((0, kc0), (1, kc1)):
                    for h in range(HPC):
                        hs = slice(h * DH, (h + 1) * DH)
                        nc.tensor.matmul(
                            sc[h][:, j * QT:(j + 1) * QT],
                            KTs[hs, kc * KC:(kc + 1) * KC],
                            QTs[hs, qsl],
                            start=True, stop=True,
                        )
                for h in range(HPC):
                    et = et_pool.tile([P, 2 * QT], BF16, tag="et")
                    nc.scalar.activation(
                        et[:], sc[h][:],
                        mybir.ActivationFunctionType.Exp,
                        scale=0.125,
                    )
                    for j, kc in ((0, kc0), (1, kc1)):
                        nc.tensor.matmul(
                            pv[h][:],
                            Vnat[h][:, kc, 0:DH + 1],
                            et[:, j * QT:(j + 1) * QT],
                            start=(kc == 0), stop=(kc == N_KC - 1),
                        )
                # previous q-tile's output projection rides in PE slack
                if qt > 0 and kcp < 8:
                    outproj_block(qt - 1, kcp)
            # normalize: attnT[h, qsl] = pv[0:DH] * (1/pv[DH])
            for h in range(HPC):
                hs = slice(h * DH, (h + 1) * DH)
                den = small.tile([1, QT], F32, tag="den")
                nc.vector.tensor_copy(den[:], pv[h][DH:DH + 1, :])
                rec = small.tile([1, QT], F32, tag="rec")
                nc.vector.reciprocal_approx_fast(rec[:], den[:])
                bc = small.tile([DH, QT], F32, tag="bc")
                nc.gpsimd.partition_broadcast(bc[:], rec[:])
                nc.vector.tensor_mul(attnT[hs, qsl], pv[h][0:DH, :], bc[:])
        for blk in range(8):
            outproj_block(N_ST - 1, blk)

        if dbg:
            for name, t in (("qts", QTs), ("kts", KTs), ("vts", VTs),
                            ("attnT", attnT)):
                nc.sync.dma_start(dbg_d[name][:, :], t[:])
            nc.sync.dma_start(dbg_d["vn0"][:, :], Vnat[0][:])
            nc.sync.dma_start(dbg_d["vn1"][:, :], Vnat[1][:])

    nc.compile()
    _CACHE[key] = nc
    return nc


def _prep_in_maps(query, key, value, mask, Wq, bq, Wk, bk, Wv, bv, Wo, bo):
    f = np.float32
    qT = np.asarray(query, dtype=f)[0].T.astype(BF16_NP)
    kT = np.asarray(key, dtype=f)[0].T.astype(BF16_NP)
    vT = np.asarray(value, dtype=f)[0].T.astype(BF16_NP)
    mm = (np.asarray(mask)[0] != 0).astype(f)
    mm = np.ascontiguousarray(mm.reshape(N_KC, KC).T)  # [128, 32]
    WqT, WkT, WvT, WoT = (np.asarray(W, dtype=f).T.astype(BF16_NP)
                          for W in (Wq, Wk, Wv, Wo))
    in_maps = []
    for c in range(NCORES):
        cs = slice(c * DSL, (c + 1) * DSL)
        in_maps.append({
            "xqT": qT, "xkT": kT, "xvT": vT,
            "wqT": np.ascontiguousarray(WqT[:, cs]).reshape(N_KCH, P, DSL),
            "wkT": np.ascontiguousarray(WkT[:, cs]).reshape(N_KCH, P, DSL),
            "wvT": np.ascontiguousarray(WvT[:, cs]).reshape(N_KCH, P, DSL),
            "woT": np.ascontiguousarray(WoT[cs, :]),
            "bq": np.ascontiguousarray(bq[cs].astype(f, copy=False)).reshape(DSL, 1),
            "bk": np.ascontiguousarray(bk[cs].astype(f, copy=False)).reshape(DSL, 1),
            "bv": np.ascontiguousarray(bv[cs].astype(f, copy=False)).reshape(DSL, 1),
            "mmul": mm,
        })
    return in_maps


def run(inputs, trace=False, trace_kwargs=None, dbg=False):
    nc = _build_nc(dbg=dbg)
    in_maps = _prep_in_maps(**inputs)
    res = run_bass_kernel_spmd(
        nc, in_maps, core_ids=list(range(NCORES)), trace=trace,
        **(trace_kwargs or {}),
    )
    bo = np.asarray(inputs["bo"], dtype=np.float32)
    acc = np.zeros((S, D), dtype=np.float32)
    for r in res.results:
        acc += r["out"]
    out = (acc + bo[None, :]).astype(np.float32)[None]
    return out, res


def kernel(**inputs):
    out, _ = run(inputs, trace=False)
    return out


# revision 26
# speedup vs baseline: 1.8906x; 1.0161x over previous
"""GroupedQueryAttention (B=1, S=4096, D=1024, G=16 heads, DH=64) on 8 TRN2 NeuronCores.

Sharding: tensor-parallel over heads. Core c computes heads {2c, 2c+1}:
  - Q/K/V projections with column-sliced weights (128 out-dims per core),
    producing Q^T/K^T/V^T in [dout, seq] layout (host pre-transposes inputs).
    Inputs stream in as 512 KB half-row chunks on two HWDGE queues (sync for
    K/V, scalar for Q) so the load runs near HBM line rate. All attention-path
    matmuls are bf16 (fp32 PSUM accumulation); bias adds run on VectorE.
  - Flash-style attention without max-subtraction (scores are tiny:
    |s/8| < ~3). The two heads' QK^T matmuls are row-packed (K=64 each,
    tile_position rows 0-63 / 64-127) so they run concurrently on the PE.
  - exp on ScalarE over two PSUM banks at once (N=1024) with fused 1/8
    scale. The attention mask is folded multiplicatively into V-natural
    (V rows and the appended ones-column are zeroed for masked keys), so
    exp needs no per-chunk bias and softmax stays exact for any 0/1 mask.
  - Softmax denominator comes free via the ones-column appended to V in the
    PV matmul (PSUM row 64 accumulates sum_k exp).
  - Output projection with row-sliced Wo produces a partial (4096, 1024)
    output per core; each q-tile's output projection is deferred into the
    next q-tile's QK/PV stream so it fills PE slack instead of stalling
    ScalarE at tile boundaries. Host sums the 8 partials and adds bo.
"""

import os
import sys

for _p in ("/opt/trn_rl_repo", "/root/.axon_site/_ro/trn_rl_repo"):
    if os.path.isdir(_p) and _p not in sys.path:
        sys.path.insert(0, _p)

from contextlib import ExitStack

import numpy as np
import ml_dtypes

import concourse.bass as bass
import concourse.mybir as mybir
import concourse.tile as tile
from concourse import bacc
from concourse.bass_utils import run_bass_kernel_spmd
from concourse.masks import make_identity

S = 4096          # sequence length
D = 1024          # model dim
G = 16            # heads
DH = 64           # head dim
P = 128           # partitions
QT = 512          # q-tile (moving free dim)
KC = 128          # k-chunk
NCORES = 8
HPC = G // NCORES             # heads per core = 2
N_ST = S // QT                # 8 s-tiles of 512
N_KCH = D // P                # 8 contraction chunks for projections
N_KC = S // KC                # 32 k-chunks for attention
DSL = P                       # per-core dout slice (2 heads * 64)
XH = S // 2                   # input DMA chunk width (half row)

F32 = mybir.dt.float32
BF16 = mybir.dt.bfloat16
BF16_NP = ml_dtypes.bfloat16

_CACHE = {}


def _build_nc(dbg=False):
    key = ("nc", dbg)
    if key in _CACHE:
        return _CACHE[key]

    nc = bacc.Bacc(
        "TRN2", target_bir_lowering=False, debug=False, num_devices=NCORES
    )

    xqT = nc.dram_tensor("xqT", [D, S], BF16, kind="ExternalInput").ap()
    xkT = nc.dram_tensor("xkT", [D, S], BF16, kind="ExternalInput").ap()
    xvT = nc.dram_tensor("xvT", [D, S], BF16, kind="ExternalInput").ap()
    wqT = nc.dram_tensor("wqT", [N_KCH, P, DSL], BF16, kind="ExternalInput").ap()
    wkT = nc.dram_tensor("wkT", [N_KCH, P, DSL], BF16, kind="ExternalInput").ap()
    wvT = nc.dram_tensor("wvT", [N_KCH, P, DSL], BF16, kind="ExternalInput").ap()
    woT = nc.dram_tensor("woT", [DSL, D], BF16, kind="ExternalInput").ap()
    bq = nc.dram_tensor("bq", [DSL, 1], F32, kind="ExternalInput").ap()
    bk = nc.dram_tensor("bk", [DSL, 1], F32, kind="ExternalInput").ap()
    bv = nc.dram_tensor("bv", [DSL, 1], F32, kind="ExternalInput").ap()
    # multiplicative 0/1 mask, [key-in-chunk, chunk]
    mmul = nc.dram_tensor("mmul", [P, N_KC], F32, kind="ExternalInput").ap()
    out_d = nc.dram_tensor("out", [S, D], F32, kind="ExternalOutput").ap()
    if dbg:
        dbg_d = {
            n: nc.dram_tensor(f"dbg_{n}", shp, dt, kind="ExternalOutput").ap()
            for n, shp, dt in (
                ("qts", [P, S], BF16), ("kts", [P, S], BF16),
                ("vts", [P, S], F32),
                ("vn0", [P, N_KC * (DH + 4)], BF16),
                ("vn1", [P, N_KC * (DH + 4)], BF16),
                ("attnT", [P, S], BF16),
            )
        }

    with tile.TileContext(nc) as tc, ExitStack() as ctx:
        consts = ctx.enter_context(tc.tile_pool(name="consts", bufs=1))
        big = ctx.enter_context(tc.tile_pool(name="big", bufs=1))
        xkv_p = ctx.enter_context(tc.tile_pool(name="xkv", bufs=16))
        xq_p = ctx.enter_context(tc.tile_pool(name="xq", bufs=8))
        et_pool = ctx.enter_context(tc.tile_pool(name="et", bufs=6))
        small = ctx.enter_context(tc.tile_pool(name="small", bufs=2))
        oevict = ctx.enter_context(tc.tile_pool(name="oevict", bufs=4))
        # single 3-deep rotation of 2-bank PSUM slots serves projections,
        # V transposes, QK scores, and the output projection (6 banks);
        # the remaining 2 banks hold the two heads' PV accumulators.
        ps_qk = ctx.enter_context(tc.tile_pool(name="ps_qk", bufs=3, space="PSUM"))
        ps_pv = ctx.enter_context(tc.tile_pool(name="ps_pv", bufs=2, space="PSUM"))

        # ---- constants ----
        ident = consts.tile([P, P], F32)
        make_identity(nc, ident[:])

        w_s = {}
        for name, wd in (("k", wkT), ("q", wqT), ("v", wvT)):
            w = consts.tile([P, N_KCH * DSL], BF16, tag=f"w{name}")
            for kc in range(N_KCH):
                nc.sync.dma_start(w[:, kc * DSL:(kc + 1) * DSL], wd[kc])
            w_s[name] = w
        wo_s = consts.tile([DSL, D], BF16, tag="wo")
        nc.sync.dma_start(wo_s[:], woT)
        b_s = {}
        for name, bd in (("q", bq), ("k", bk), ("v", bv)):
            b = consts.tile([DSL, 1], F32, tag=f"b{name}")
            nc.sync.dma_start(b[:], bd)
            b_s[name] = b
        mm_s = consts.tile([P, N_KC], F32, tag="mm")
        nc.sync.dma_start(mm_s[:], mmul)

        # ---- resident activations ----
        QTs = big.tile([P, S], BF16, tag="QTs")      # Q^T  [dout, s]
        KTs = big.tile([P, S], BF16, tag="KTs")      # K^T  [dout, s]
        VTs = big.tile([P, S], F32, tag="VTs")       # V^T  [dout, s]
        # V natural per head: [k-part, chunk, DH+1]; col DH is the mask
        # column (1/0) that makes PV also accumulate sum_k exp (softmax
        # denom) restricted to unmasked keys.
        Vnat = [
            big.tile([P, N_KC, DH + 4], BF16, tag=f"Vn{h}", name=f"Vnat{h}")
            for h in range(HPC)
        ]
        attnT = big.tile([P, S], BF16, tag="attnT")  # normalized attn^T [din, s]

        for h in range(HPC):
            # mask column: exactly the 0/1 mask per key
            nc.vector.tensor_copy(Vnat[h][:, :, DH], mm_s[:])

        # ---- phase 1: input DMA + projections ----
        # V first (split across both HWDGE queues) so PV never waits; then
        # K on sync || Q on scalar. 512 KB half-row chunks hit near HBM
        # line rate. Only K/Q half 0 projections run before attention; the
        # half-1 projections are interleaved into early attention steps so
        # the PE never idles waiting for their DMAs.
        def load_half(name, xd, half, engines):
            hsl = slice(half * XH, (half + 1) * XH)
            pool = xq_p if name == "q" else xkv_p
            xt = []
            for kc in range(N_KCH):
                x = pool.tile([P, XH], BF16, tag="xf", name=f"x{name}{kc}")
                engines[kc % len(engines)].dma_start(
                    x[:], xd[kc * P:(kc + 1) * P, hsl]
                )
                xt.append(x)
            return xt

        def proj_stile(name, xt, half, j, evict):
            st = half * (XH // QT) + j
            sl = slice(st * QT, (st + 1) * QT)
            jsl = slice(j * QT, (j + 1) * QT)
            ps = ps_qk.tile([P, QT], F32, tag="qk", name="ps")
            for kc in range(N_KCH):
                nc.tensor.matmul(
                    ps[:],
                    w_s[name][:, kc * DSL:(kc + 1) * DSL],
                    xt[kc][:, jsl],
                    start=(kc == 0),
                    stop=(kc == N_KCH - 1),
                )
            evict(ps, sl, st)

        def evict_k(ps, sl, st):
            nc.vector.tensor_scalar_add(KTs[:, sl], ps[:], b_s["k"][:])

        def evict_q(ps, sl, st):
            nc.vector.tensor_scalar_add(QTs[:, sl], ps[:], b_s["q"][:])

        def evict_v(ps, sl, st):
            nc.vector.tensor_scalar_add(VTs[:, sl], ps[:], b_s["v"][:])
            # transpose this s-tile of V^T into V natural (4 k-chunks),
            # applying the multiplicative key mask
            for h in range(HPC):
                hs = slice(h * DH, (h + 1) * DH)
                for jj in range(QT // KC):
                    kc = st * (QT // KC) + jj
                    pt = ps_qk.tile([P, DH], F32, tag="qk", name="pt")
                    nc.tensor.transpose(
                        pt[:], VTs[hs, kc * KC:(kc + 1) * KC], ident[hs, hs]
                    )
                    nc.vector.tensor_mul(
                        Vnat[h][:, kc, 0:DH],
                        pt[:],
                        mm_s[:, kc:kc + 1].to_broadcast((P, DH)),
                    )

        # V: both queues, projected fully up front
        for half in range(2):
            xt = load_half("v", xvT, half, [nc.sync, nc.scalar])
            for j in range(XH // QT):
                proj_stile("v", xt, half, j, evict_v)
        # K on sync, Q on scalar, concurrently; half 0 projected now,
        # half 1 deferred into the attention stream
        xk = [load_half("k", xkT, 0, [nc.sync])]
        xq = [load_half("q", xqT, 0, [nc.scalar])]
        for j in range(XH // QT):
            proj_stile("k", xk[0], 0, j, evict_k)
        for j in range(XH // QT):
            proj_stile("q", xq[0], 0, j, evict_q)
        xk.append(load_half("k", xkT, 1, [nc.sync]))
        xq.append(load_half("q", xqT, 1, [nc.scalar]))

        # deferred work items, drained one per attention step:
        # (qt index at which it may run, emit-callback)
        deferred = []
        for j in range(XH // QT):
            deferred.append((0, lambda j=j: proj_stile("k", xk[1], 1, j, evict_k)))
        for j in range(XH // QT):
            deferred.append((1, lambda j=j: proj_stile("q", xq[1], 1, j, evict_q)))

        # ---- phase 2+3: attention, software-pipelined ----
        def outproj_block(qt, blk):
            st = qt * (QT // P) + blk // 2
            nt = blk % 2
            po = ps_qk.tile([P, QT], F32, tag="qk", name="po")
            nc.tensor.matmul(
                po[:],
                attnT[:, st * P:(st + 1) * P],
                wo_s[:, nt * QT:(nt + 1) * QT],
                start=True, stop=True,
            )
            ot = oevict.tile([P, QT], F32, tag="ot")
            nc.vector.tensor_copy(ot[:], po[:])
            nc.sync.dma_start(
                out_d[st * P:(st + 1) * P, nt * QT:(nt + 1) * QT], ot[:]
            )

        def emit_pv(pv, ets, kc):
            et = ets.pop(kc)
            for h in range(HPC):
                nc.tensor.matmul(
                    pv[h][:],
                    Vnat[h][:, kc, 0:DH + 1],
                    et[:, h * QT:(h + 1) * QT],
                    start=(kc == 0), stop=(kc == N_KC - 1),
                )

        for qt in range(N_ST):
            qsl = slice(qt * QT, (qt + 1) * QT)
            pv = [
                ps_pv.tile([DH + 1, QT], F32, tag="pv", name=f"pv{h}")
                for h in range(HPC)
            ]
            ets = {}
            for kc in range(N_KC):
                # both heads' QK^T into one 2-bank tile, adjacent emission
                # so the K=64 matmuls row-pack and run concurrently
                sc = ps_qk.tile([P, 2 * QT], F32, tag="qk", name="sc")
                for h in range(HPC):
                    hs = slice(h * DH, (h + 1) * DH)
                    nc.tensor.matmul(
                        sc[:, h * QT:(h + 1) * QT],
                        KTs[hs, kc * KC:(kc + 1) * KC],
                        QTs[hs, qsl],
                        start=True, stop=True,
                    )
                et = et_pool.tile([P, 2 * QT], BF16, tag="et")
                nc.scalar.activation(
                    et[:], sc[:],
                    mybir.ActivationFunctionType.Exp,
                    scale=0.125,
                )
                ets[kc] = et
                # PV delayed one step: exp(kc-1) is long done, PE never
                # stalls on ScalarE
                if kc > 0:
                    emit_pv(pv, ets, kc - 1)
                # deferred K/Q half-1 projections and the previous q-tile's
                # output projection ride in PE slack; in qt 0 they wait
                # until their DMAs have had time to land
                if (kc % 2 == 1 and (qt > 0 or kc >= 15)
                        and deferred and deferred[0][0] <= qt):
                    deferred.pop(0)[1]()
                if qt > 0 and 3 <= kc < 27 and kc % 3 == 0:
                    outproj_block(qt - 1, (kc - 3) // 3)
            emit_pv(pv, ets, N_KC - 1)
            # normalize: attnT[h, qsl] = pv[0:DH] * (1/pv[DH])
            for h in range(HPC):
                hs = slice(h * DH, (h + 1) * DH)
                den = small.tile([1, QT], F32, tag="den")
                nc.vector.tensor_copy(den[:], pv[h][DH:DH + 1, :])
                rec = small.tile([1, QT], F32, tag="rec")
                nc.vector.reciprocal_approx_fast(rec[:], den[:])
                bc = small.tile([DH, QT], F32, tag="bc")
                nc.gpsimd.partition_broadcast(bc[:], rec[:])
                nc.vector.tensor_mul(attnT[hs, qsl], pv[h][0:DH, :], bc[:])
        for blk in range(8):
            outproj_block(N_ST - 1, blk)

        if dbg:
            for name, t in (("qts", QTs), ("kts", KTs), ("vts", VTs),
                            ("attnT", attnT)):
                nc.sync.dma_start(dbg_d[name][:, :], t[:])
            nc.sync.dma_start(dbg_d["vn0"][:, :], Vnat[0][:])
            nc.sync.dma_start(dbg_d["vn1"][:, :], Vnat[1][:])

    nc.compile()
    _CACHE[key] = nc
    return nc


def _prep_in_maps(query, key, value, mask, Wq, bq, Wk, bk, Wv, bv, Wo, bo):
    f = np.float32
    qT = np.asarray(query, dtype=f)[0].T.astype(BF16_NP)
    kT = np.asarray(key, dtype=f)[0].T.astype(BF16_NP)
    vT = np.asarray(value, dtype=f)[0].T.astype(BF16_NP)
    mm = (np.asarray(mask)[0] != 0).astype(f)
    mm = np.ascontiguousarray(mm.reshape(N_KC, KC).T)  # [128, 32]
    WqT, WkT, WvT, WoT = (np.asarray(W, dtype=f).T.astype(BF16_NP)
                          for W in (Wq, Wk, Wv, Wo))
    in_maps = []
    for c in range(NCORES):
        cs = slice(c * DSL, (c + 1) * DSL)
        in_maps.append({
            "xqT": qT, "xkT": kT, "xvT": vT,
            "wqT": np.ascontiguousarray(WqT[:, cs]).reshape(N_KCH, P, DSL),
            "wkT": np.ascontiguousarray(WkT[:, cs]).reshape(N_KCH, P, DSL),
            "wvT": np.ascontiguousarray(WvT[:, cs]).reshape(N_KCH, P, DSL),
            "woT": np.ascontiguousarray(WoT[cs, :]),
            "bq": np.ascontiguousarray(bq[cs].astype(f, copy=False)).reshape(DSL, 1),
            "bk": np.ascontiguousarray(bk[cs].astype(f, copy=False)).reshape(DSL, 1),
            "bv": np.ascontiguousarray(bv[cs].astype(f, copy=False)).reshape(DSL, 1),
            "mmul": mm,
        })
    return in_maps


def run(inputs, trace=False, trace_kwargs=None, dbg=False):
    nc = _build_nc(dbg=dbg)
    in_maps = _prep_in_maps(**inputs)
    res = run_bass_kernel_spmd(
        nc, in_maps, core_ids=list(range(NCORES)), trace=trace,
        **(trace_kwargs or {}),
    )
    bo = np.asarray(inputs["bo"], dtype=np.float32)
    acc = np.zeros((S, D), dtype=np.float32)
    for r in res.results:
        acc += r["out"]
    out = (acc + bo[None, :]).astype(np.float32)[None]
    return out, res


def kernel(**inputs):
    out, _ = run(inputs, trace=False)
    return out


# revision 32
# speedup vs baseline: 2.3635x; 1.2501x over previous
"""GroupedQueryAttention (B=1, S=4096, D=1024, G=16 heads, DH=64) on 8 TRN2 NeuronCores.

Sharding: tensor-parallel over heads. Core c computes heads {2c, 2c+1}:
  - Q/K/V projections with column-sliced weights (128 out-dims per core),
    producing Q^T/K^T/V^T in [dout, seq] layout (host pre-transposes inputs).
    Inputs stream in as 512 KB half-row chunks on two HWDGE queues (sync for
    K/V, scalar for Q) so the load runs near HBM line rate. All attention-path
    matmuls are bf16 (fp32 PSUM accumulation); bias adds run on VectorE.
  - Flash-style attention without max-subtraction (scores are tiny:
    |s/8| < ~3). The two heads' QK^T matmuls are row-packed (K=64 each,
    tile_position rows 0-63 / 64-127) so they run concurrently on the PE.
  - exp on ScalarE over two PSUM banks at once (N=1024) with fused 1/8
    scale. The attention mask is folded multiplicatively into V-natural
    (V rows and the appended ones-column are zeroed for masked keys), so
    exp needs no per-chunk bias and softmax stays exact for any 0/1 mask.
  - Softmax denominator comes free via the ones-column appended to V in the
    PV matmul (PSUM row 64 accumulates sum_k exp).
  - Output projection with row-sliced Wo produces a partial (4096, 1024)
    output per core; each q-tile's output projection is deferred into the
    next q-tile's QK/PV stream so it fills PE slack instead of stalling
    ScalarE at tile boundaries. Host sums the 8 partials and adds bo.
"""

import os
import sys

for _p in ("/opt/trn_rl_repo", "/root/.axon_site/_ro/trn_rl_repo"):
    if os.path.isdir(_p) and _p not in sys.path:
        sys.path.insert(0, _p)

from contextlib import ExitStack

import numpy as np
import ml_dtypes

import concourse.bass as bass
import concourse.mybir as mybir
import concourse.tile as tile
from concourse import bacc
from concourse.bass_utils import run_bass_kernel_spmd
from concourse.masks import make_identity

S = 4096          # sequence length
D = 1024          # model dim
G = 16            # heads
DH = 64           # head dim
P = 128           # partitions
QT = 512          # q-tile (moving free dim)
KC = 128          # k-chunk
NCORES = 8
HPC = G // NCORES             # heads per core = 2
N_ST = S // QT                # 8 s-tiles of 512
N_KCH = D // P                # 8 contraction chunks for projections
N_KC = S // KC                # 32 k-chunks for attention
DSL = P                       # per-core dout slice (2 heads * 64)
XH = S // 2                   # input DMA chunk width (half row)

F32 = mybir.dt.float32
BF16 = mybir.dt.bfloat16
BF16_NP = ml_dtypes.bfloat16

_CACHE = {}


def _build_nc(dbg=False):
    key = ("nc", dbg)
    if key in _CACHE:
        return _CACHE[key]

    nc = bacc.Bacc(
        "TRN2", target_bir_lowering=False, debug=False, num_devices=NCORES
    )

    xqT = nc.dram_tensor("xqT", [D, S], BF16, kind="ExternalInput").ap()
    xkT = nc.dram_tensor("xkT", [D, S], BF16, kind="ExternalInput").ap()
    xvT = nc.dram_tensor("xvT", [D, S], BF16, kind="ExternalInput").ap()
    wqT = nc.dram_tensor("wqT", [N_KCH, P, DSL], BF16, kind="ExternalInput").ap()
    wkT = nc.dram_tensor("wkT", [N_KCH, P, DSL], BF16, kind="ExternalInput").ap()
    wvT = nc.dram_tensor("wvT", [N_KCH, P, DSL], BF16, kind="ExternalInput").ap()
    woT = nc.dram_tensor("woT", [DSL, D], BF16, kind="ExternalInput").ap()
    bq = nc.dram_tensor("bq", [DSL, 1], F32, kind="ExternalInput").ap()
    bk = nc.dram_tensor("bk", [DSL, 1], F32, kind="ExternalInput").ap()
    bv = nc.dram_tensor("bv", [DSL, 1], F32, kind="ExternalInput").ap()
    # multiplicative 0/1 mask, [key-in-chunk, chunk]
    mmul = nc.dram_tensor("mmul", [P, N_KC], F32, kind="ExternalInput").ap()
    out_d = nc.dram_tensor("out", [S, D], F32, kind="ExternalOutput").ap()
    if dbg:
        dbg_d = {
            n: nc.dram_tensor(f"dbg_{n}", shp, dt, kind="ExternalOutput").ap()
            for n, shp, dt in (
                ("qts", [P, S], BF16), ("kts", [P, S], BF16),
                ("vts", [P, S], F32),
                ("vn0", [P, N_KC * (DH + 4)], BF16),
                ("vn1", [P, N_KC * (DH + 4)], BF16),
                ("attnT", [P, S], BF16),
            )
        }

    with tile.TileContext(nc) as tc, ExitStack() as ctx:
        consts = ctx.enter_context(tc.tile_pool(name="consts", bufs=1))
        big = ctx.enter_context(tc.tile_pool(name="big", bufs=1))
        xkv_p = ctx.enter_context(tc.tile_pool(name="xkv", bufs=16))
        xq_p = ctx.enter_context(tc.tile_pool(name="xq", bufs=8))
        et_pool = ctx.enter_context(tc.tile_pool(name="et", bufs=6))
        small = ctx.enter_context(tc.tile_pool(name="small", bufs=2))
        oevict = ctx.enter_context(tc.tile_pool(name="oevict", bufs=4))
        # single 3-deep rotation of 2-bank PSUM slots serves projections,
        # V transposes, QK scores, and the output projection (6 banks);
        # the remaining 2 banks hold the two heads' PV accumulators.
        ps_qk = ctx.enter_context(tc.tile_pool(name="ps_qk", bufs=3, space="PSUM"))
        ps_pv = ctx.enter_context(tc.tile_pool(name="ps_pv", bufs=2, space="PSUM"))

        # ---- constants ----
        ident = consts.tile([P, P], F32)
        make_identity(nc, ident[:])

        # weights/biases load via the gpsimd SWDGE queue so they don't
        # head-of-line block the input streams on the two HWDGE queues
        w_s = {}
        for name, wd in (("k", wkT), ("q", wqT), ("v", wvT)):
            w = consts.tile([P, N_KCH * DSL], BF16, tag=f"w{name}")
            for kc in range(N_KCH):
                nc.gpsimd.dma_start(w[:, kc * DSL:(kc + 1) * DSL], wd[kc])
            w_s[name] = w
        wo_s = consts.tile([DSL, D], BF16, tag="wo")
        nc.gpsimd.dma_start(wo_s[:], woT)
        b_s = {}
        for name, bd in (("q", bq), ("k", bk), ("v", bv)):
            b = consts.tile([DSL, 1], F32, tag=f"b{name}")
            nc.gpsimd.dma_start(b[:], bd)
            b_s[name] = b
        mm_s = consts.tile([P, N_KC], F32, tag="mm")
        nc.gpsimd.dma_start(mm_s[:], mmul)

        # ---- resident activations ----
        QTs = big.tile([P, S], BF16, tag="QTs")      # Q^T  [dout, s]
        KTs = big.tile([P, S], BF16, tag="KTs")      # K^T  [dout, s]
        VTs = big.tile([P, S], F32, tag="VTs")       # V^T  [dout, s]
        # V natural per head: [k-part, chunk, DH+1]; col DH is the mask
        # column (1/0) that makes PV also accumulate sum_k exp (softmax
        # denom) restricted to unmasked keys.
        Vnat = [
            big.tile([P, N_KC, DH + 4], BF16, tag=f"Vn{h}", name=f"Vnat{h}")
            for h in range(HPC)
        ]
        attnT = big.tile([P, S], BF16, tag="attnT")  # normalized attn^T [din, s]

        for h in range(HPC):
            # mask column: exactly the 0/1 mask per key
            nc.vector.tensor_copy(Vnat[h][:, :, DH], mm_s[:])

        # ---- phase 1: input DMA + projections ----
        # V first (split across both HWDGE queues) so PV never waits; then
        # K on sync || Q on scalar. 512 KB half-row chunks hit near HBM
        # line rate. Only K/Q half 0 projections run before attention; the
        # half-1 projections are interleaved into early attention steps so
        # the PE never idles waiting for their DMAs.
        def load_half(name, xd, half, engines):
            hsl = slice(half * XH, (half + 1) * XH)
            pool = xq_p if name == "q" else xkv_p
            xt = []
            for kc in range(N_KCH):
                x = pool.tile([P, XH], BF16, tag="xf", name=f"x{name}{kc}")
                engines[kc % len(engines)].dma_start(
                    x[:], xd[kc * P:(kc + 1) * P, hsl]
                )
                xt.append(x)
            return xt

        def proj_stile(name, xt, half, j, evict):
            st = half * (XH // QT) + j
            sl = slice(st * QT, (st + 1) * QT)
            jsl = slice(j * QT, (j + 1) * QT)
            ps = ps_qk.tile([P, QT], F32, tag="qk", name="ps")
            for kc in range(N_KCH):
                nc.tensor.matmul(
                    ps[:],
                    w_s[name][:, kc * DSL:(kc + 1) * DSL],
                    xt[kc][:, jsl],
                    start=(kc == 0),
                    stop=(kc == N_KCH - 1),
                )
            evict(ps, sl, st)

        def evict_k(ps, sl, st):
            nc.vector.tensor_scalar_add(KTs[:, sl], ps[:], b_s["k"][:])

        def evict_q(ps, sl, st):
            nc.vector.tensor_scalar_add(QTs[:, sl], ps[:], b_s["q"][:])

        def evict_v(ps, sl, st):
            nc.vector.tensor_scalar_add(VTs[:, sl], ps[:], b_s["v"][:])
            # transpose this s-tile of V^T into V natural (4 k-chunks),
            # applying the multiplicative key mask
            for h in range(HPC):
                hs = slice(h * DH, (h + 1) * DH)
                for jj in range(QT // KC):
                    kc = st * (QT // KC) + jj
                    pt = ps_qk.tile([P, DH], F32, tag="qk", name="pt")
                    nc.tensor.transpose(
                        pt[:], VTs[hs, kc * KC:(kc + 1) * KC], ident[hs, hs]
                    )
                    nc.vector.tensor_mul(
                        Vnat[h][:, kc, 0:DH],
                        pt[:],
                        mm_s[:, kc:kc + 1].to_broadcast((P, DH)),
                    )

        # DMA issue order: K half 0 (sync) || Q half 0 (scalar) lead so the
        # first QK can fire as early as possible; V follows on both queues;
        # K/Q half 1 trail. PE order: K/Q half-0 projections, V projections,
        # then attention with the half-1 projections drained into PE slack.
        xk = [load_half("k", xkT, 0, [nc.sync])]
        xq = [load_half("q", xqT, 0, [nc.scalar])]
        xv = [load_half("v", xvT, 0, [nc.sync, nc.scalar])]
        xv.append(load_half("v", xvT, 1, [nc.sync, nc.scalar]))
        xk.append(load_half("k", xkT, 1, [nc.sync]))
        xq.append(load_half("q", xqT, 1, [nc.scalar]))

        for j in range(XH // QT):
            proj_stile("k", xk[0], 0, j, evict_k)
        for j in range(XH // QT):
            proj_stile("q", xq[0], 0, j, evict_q)
        for j in range(XH // QT):
            proj_stile("v", xv[0], 0, j, evict_v)

        # deferred work items, drained one per attention step:
        # (qt index at which it may run, emit-callback). V half 1 first
        # (PV needs chunk 16 at step 17), then K half 1 (QK needs chunk 16
        # at step 16 only after its s-tile drained), Q half 1 during qt 1.
        deferred = []
        for j in range(XH // QT):
            deferred.append((0, lambda j=j: proj_stile("v", xv[1], 1, j, evict_v)))
        for j in range(XH // QT):
            deferred.append((0, lambda j=j: proj_stile("k", xk[1], 1, j, evict_k)))
        for j in range(XH // QT):
            deferred.append((1, lambda j=j: proj_stile("q", xq[1], 1, j, evict_q)))

        # ---- phase 2+3: attention, software-pipelined ----
        def outproj_block(qt, blk):
            st = qt * (QT // P) + blk // 2
            nt = blk % 2
            po = ps_qk.tile([P, QT], F32, tag="qk", name="po")
            nc.tensor.matmul(
                po[:],
                attnT[:, st * P:(st + 1) * P],
                wo_s[:, nt * QT:(nt + 1) * QT],
                start=True, stop=True,
            )
            ot = oevict.tile([P, QT], F32, tag="ot")
            nc.vector.tensor_copy(ot[:], po[:])
            nc.sync.dma_start(
                out_d[st * P:(st + 1) * P, nt * QT:(nt + 1) * QT], ot[:]
            )

        def emit_pv(pv, ets, kc):
            et = ets.pop(kc)
            for h in range(HPC):
                nc.tensor.matmul(
                    pv[h][:],
                    Vnat[h][:, kc, 0:DH + 1],
                    et[:, h * QT:(h + 1) * QT],
                    start=(kc == 0), stop=(kc == N_KC - 1),
                )

        for qt in range(N_ST):
            qsl = slice(qt * QT, (qt + 1) * QT)
            pv = [
                ps_pv.tile([DH + 1, QT], F32, tag="pv", name=f"pv{h}")
                for h in range(HPC)
            ]
            ets = {}
            for kc in range(N_KC):
                # both heads' QK^T into one 2-bank tile, adjacent emission
                # so the K=64 matmuls row-pack and run concurrently
                sc = ps_qk.tile([P, 2 * QT], F32, tag="qk", name="sc")
                for h in range(HPC):
                    hs = slice(h * DH, (h + 1) * DH)
                    nc.tensor.matmul(
                        sc[:, h * QT:(h + 1) * QT],
                        KTs[hs, kc * KC:(kc + 1) * KC],
                        QTs[hs, qsl],
                        start=True, stop=True,
                    )
                et = et_pool.tile([P, 2 * QT], BF16, tag="et")
                nc.scalar.activation(
                    et[:], sc[:],
                    mybir.ActivationFunctionType.Exp,
                    scale=0.125,
                )
                ets[kc] = et
                # PV delayed one step: exp(kc-1) is long done, PE never
                # stalls on ScalarE
                if kc > 0:
                    emit_pv(pv, ets, kc - 1)
                # deferred V/K/Q half-1 projections and the previous
                # q-tile's output projection ride in PE slack; in qt 0 they
                # wait until their DMAs have had time to land
                if (kc % 2 == 1 and (qt > 0 or kc >= 7)
                        and deferred and deferred[0][0] <= qt):
                    deferred.pop(0)[1]()
                if qt > 0 and 3 <= kc < 27 and kc % 3 == 0:
                    outproj_block(qt - 1, (kc - 3) // 3)
            emit_pv(pv, ets, N_KC - 1)
            # normalize: attnT[h, qsl] = pv[0:DH] * (1/pv[DH]).
            # The denominator copy runs on ScalarE, which is idle right at
            # the q-tile boundary, shortening the pv-slot handoff chain.
            for h in range(HPC):
                hs = slice(h * DH, (h + 1) * DH)
                den = small.tile([1, QT], F32, tag="den")
                nc.scalar.copy(den[:], pv[h][DH:DH + 1, :])
                rec = small.tile([1, QT], F32, tag="rec")
                nc.vector.reciprocal_approx_fast(rec[:], den[:])
                bc = small.tile([DH, QT], F32, tag="bc")
                nc.gpsimd.partition_broadcast(bc[:], rec[:])
                nc.vector.tensor_mul(attnT[hs, qsl], pv[h][0:DH, :], bc[:])
        for blk in range(8):
            outproj_block(N_ST - 1, blk)

        if dbg:
            for name, t in (("qts", QTs), ("kts", KTs), ("vts", VTs),
                            ("attnT", attnT)):
                nc.sync.dma_start(dbg_d[name][:, :], t[:])
            nc.sync.dma_start(dbg_d["vn0"][:, :], Vnat[0][:])
            nc.sync.dma_start(dbg_d["vn1"][:, :], Vnat[1][:])

    nc.compile()
    _CACHE[key] = nc
    return nc


def _prep_in_maps(query, key, value, mask, Wq, bq, Wk, bk, Wv, bv, Wo, bo):
    f = np.float32
    qT = np.asarray(query, dtype=f)[0].T.astype(BF16_NP)
    kT = np.asarray(key, dtype=f)[0].T.astype(BF16_NP)
    vT = np.asarray(value, dtype=f)[0].T.astype(BF16_NP)
    mm = (np.asarray(mask)[0] != 0).astype(f)
    mm = np.ascontiguousarray(mm.reshape(N_KC, KC).T)  # [128, 32]
    WqT, WkT, WvT, WoT = (np.asarray(W, dtype=f).T.astype(BF16_NP)
                          for W in (Wq, Wk, Wv, Wo))
    in_maps = []
    for c in range(NCORES):
        cs = slice(c * DSL, (c + 1) * DSL)
        in_maps.append({
            "xqT": qT, "xkT": kT, "xvT": vT,
            "wqT": np.ascontiguousarray(WqT[:, cs]).reshape(N_KCH, P, DSL),
            "wkT": np.ascontiguousarray(WkT[:, cs]).reshape(N_KCH, P, DSL),
            "wvT": np.ascontiguousarray(WvT[:, cs]).reshape(N_KCH, P, DSL),
            "woT": np.ascontiguousarray(WoT[cs, :]),
            "bq": np.ascontiguousarray(bq[cs].astype(f, copy=False)).reshape(DSL, 1),
            "bk": np.ascontiguousarray(bk[cs].astype(f, copy=False)).reshape(DSL, 1),
            "bv": np.ascontiguousarray(bv[cs].astype(f, copy=False)).reshape(DSL, 1),
            "mmul": mm,
        })
    return in_maps


def run(inputs, trace=False, trace_kwargs=None, dbg=False):
    nc = _build_nc(dbg=dbg)
    in_maps = _prep_in_maps(**inputs)
    res = run_bass_kernel_spmd(
        nc, in_maps, core_ids=list(range(NCORES)), trace=trace,
        **(trace_kwargs or {}),
    )
    bo = np.asarray(inputs["bo"], dtype=np.float32)
    acc = np.zeros((S, D), dtype=np.float32)
    for r in res.results:
        acc += r["out"]
    out = (acc + bo[None, :]).astype(np.float32)[None]
    return out, res


def kernel(**inputs):
    out, _ = run(inputs, trace=False)
    return out


# revision 39
# speedup vs baseline: 2.4039x; 1.0171x over previous
"""GroupedQueryAttention (B=1, S=4096, D=1024, G=16 heads, DH=64) on 8 TRN2 NeuronCores.

Sharding: tensor-parallel over heads. Core c computes heads {2c, 2c+1}:
  - Q/K/V projections with column-sliced weights (128 out-dims per core),
    producing Q^T/K^T/V^T in [dout, seq] layout (host pre-transposes inputs).
    Inputs stream in as 512 KB half-row chunks on two HWDGE queues (sync for
    K/V, scalar for Q) so the load runs near HBM line rate. All attention-path
    matmuls are bf16 (fp32 PSUM accumulation); bias adds run on VectorE.
  - Flash-style attention without max-subtraction (scores are tiny:
    |s/8| < ~3). The two heads' QK^T matmuls are row-packed (K=64 each,
    tile_position rows 0-63 / 64-127) so they run concurrently on the PE.
  - exp on ScalarE over two PSUM banks at once (N=1024) with fused 1/8
    scale. The attention mask is folded multiplicatively into V-natural
    (V rows and the appended ones-column are zeroed for masked keys), so
    exp needs no per-chunk bias and softmax stays exact for any 0/1 mask.
  - Softmax denominator comes free via the ones-column appended to V in the
    PV matmul (PSUM row 64 accumulates sum_k exp).
  - Output projection with row-sliced Wo produces a partial (4096, 1024)
    output per core; each q-tile's output projection is deferred into the
    next q-tile's QK/PV stream so it fills PE slack instead of stalling
    ScalarE at tile boundaries. Host sums the 8 partials and adds bo.
"""

import os
import sys

for _p in ("/opt/trn_rl_repo", "/root/.axon_site/_ro/trn_rl_repo"):
    if os.path.isdir(_p) and _p not in sys.path:
        sys.path.insert(0, _p)

from contextlib import ExitStack

import numpy as np
import ml_dtypes

import concourse.bass as bass
import concourse.mybir as mybir
import concourse.tile as tile
from concourse import bacc
from concourse.bass_utils import run_bass_kernel_spmd
from concourse.masks import make_identity

S = 4096          # sequence length
D = 1024          # model dim
G = 16            # heads
DH = 64           # head dim
P = 128           # partitions
QT = 512          # q-tile (moving free dim)
KC = 128          # k-chunk
NCORES = 8
HPC = G // NCORES             # heads per core = 2
N_ST = S // QT                # 8 s-tiles of 512
N_KCH = D // P                # 8 contraction chunks for projections
N_KC = S // KC                # 32 k-chunks for attention
DSL = P                       # per-core dout slice (2 heads * 64)
XH = S // 2                   # input DMA chunk width (half row)

F32 = mybir.dt.float32
BF16 = mybir.dt.bfloat16
BF16_NP = ml_dtypes.bfloat16

_CACHE = {}


def _build_nc(dbg=False):
    key = ("nc", dbg)
    if key in _CACHE:
        return _CACHE[key]

    nc = bacc.Bacc(
        "TRN2", target_bir_lowering=False, debug=False, num_devices=NCORES
    )

    xqT = nc.dram_tensor("xqT", [D, S], BF16, kind="ExternalInput").ap()
    xkT = nc.dram_tensor("xkT", [D, S], BF16, kind="ExternalInput").ap()
    xvT = nc.dram_tensor("xvT", [D, S], BF16, kind="ExternalInput").ap()
    wqT = nc.dram_tensor("wqT", [N_KCH, P, DSL], BF16, kind="ExternalInput").ap()
    wkT = nc.dram_tensor("wkT", [N_KCH, P, DSL], BF16, kind="ExternalInput").ap()
    wvT = nc.dram_tensor("wvT", [N_KCH, P, DSL], BF16, kind="ExternalInput").ap()
    woT = nc.dram_tensor("woT", [DSL, D], BF16, kind="ExternalInput").ap()
    bq = nc.dram_tensor("bq", [DSL, 1], F32, kind="ExternalInput").ap()
    bk = nc.dram_tensor("bk", [DSL, 1], F32, kind="ExternalInput").ap()
    bv = nc.dram_tensor("bv", [DSL, 1], F32, kind="ExternalInput").ap()
    # multiplicative 0/1 mask, [key-in-chunk, chunk]
    mmul = nc.dram_tensor("mmul", [P, N_KC], F32, kind="ExternalInput").ap()
    out_d = nc.dram_tensor("out", [S, D], F32, kind="ExternalOutput").ap()
    if dbg:
        dbg_d = {
            n: nc.dram_tensor(f"dbg_{n}", shp, dt, kind="ExternalOutput").ap()
            for n, shp, dt in (
                ("qts", [P, S], BF16), ("kts", [P, S], BF16),
                ("vts", [P, S], F32),
                ("vn0", [P, N_KC * (DH + 4)], BF16),
                ("vn1", [P, N_KC * (DH + 4)], BF16),
                ("attnT", [P, S], BF16),
            )
        }

    with tile.TileContext(nc) as tc, ExitStack() as ctx:
        consts = ctx.enter_context(tc.tile_pool(name="consts", bufs=1))
        big = ctx.enter_context(tc.tile_pool(name="big", bufs=1))
        xkv_p = ctx.enter_context(tc.tile_pool(name="xkv", bufs=12))
        xq_p = ctx.enter_context(tc.tile_pool(name="xq", bufs=16))
        et_pool = ctx.enter_context(tc.tile_pool(name="et", bufs=6))
        small = ctx.enter_context(tc.tile_pool(name="small", bufs=2))
        oevict = ctx.enter_context(tc.tile_pool(name="oevict", bufs=4))
        # single 3-deep rotation of 2-bank PSUM slots serves projections,
        # V transposes, QK scores, and the output projection (6 banks);
        # the remaining 2 banks hold the two heads' PV accumulators.
        ps_qk = ctx.enter_context(tc.tile_pool(name="ps_qk", bufs=3, space="PSUM"))
        ps_pv = ctx.enter_context(tc.tile_pool(name="ps_pv", bufs=2, space="PSUM"))

        # ---- constants ----
        ident = consts.tile([P, P], F32)
        make_identity(nc, ident[:])

        # weights/biases load via the gpsimd SWDGE queue so they don't
        # head-of-line block the input streams on the two HWDGE queues;
        # tiny biases/mask first since projection evicts gate on them
        b_s = {}
        for name, bd in (("q", bq), ("k", bk), ("v", bv)):
            b = consts.tile([DSL, 1], F32, tag=f"b{name}")
            nc.gpsimd.dma_start(b[:], bd)
            b_s[name] = b
        mm_s = consts.tile([P, N_KC], F32, tag="mm")
        nc.gpsimd.dma_start(mm_s[:], mmul)
        w_s = {}
        for name, wd in (("k", wkT), ("q", wqT), ("v", wvT)):
            w = consts.tile([P, N_KCH * DSL], BF16, tag=f"w{name}")
            for kc in range(N_KCH):
                nc.gpsimd.dma_start(w[:, kc * DSL:(kc + 1) * DSL], wd[kc])
            w_s[name] = w
        wo_s = consts.tile([DSL, D], BF16, tag="wo")
        nc.gpsimd.dma_start(wo_s[:], woT)

        # ---- resident activations ----
        QTs = big.tile([P, S], BF16, tag="QTs")      # Q^T  [dout, s]
        KTs = big.tile([P, S], BF16, tag="KTs")      # K^T  [dout, s]
        VTs = big.tile([P, S], F32, tag="VTs")       # V^T  [dout, s]
        # V natural per head: [k-part, chunk, DH+1]; col DH is the mask
        # column (1/0) that makes PV also accumulate sum_k exp (softmax
        # denom) restricted to unmasked keys.
        Vnat = [
            big.tile([P, N_KC, DH + 4], BF16, tag=f"Vn{h}", name=f"Vnat{h}")
            for h in range(HPC)
        ]
        attnT = big.tile([P, S], BF16, tag="attnT")  # normalized attn^T [din, s]

        for h in range(HPC):
            # mask column: exactly the 0/1 mask per key
            nc.vector.tensor_copy(Vnat[h][:, :, DH], mm_s[:])

        # ---- phase 1: input DMA + projections ----
        # V first (split across both HWDGE queues) so PV never waits; then
        # K on sync || Q on scalar. 512 KB half-row chunks hit near HBM
        # line rate. Only K/Q half 0 projections run before attention; the
        # half-1 projections are interleaved into early attention steps so
        # the PE never idles waiting for their DMAs.
        def load_half(name, xd, half, engines, pool):
            hsl = slice(half * XH, (half + 1) * XH)
            xt = []
            for kc in range(N_KCH):
                x = pool.tile([P, XH], BF16, tag="xf", name=f"x{name}{kc}")
                engines[kc % len(engines)].dma_start(
                    x[:], xd[kc * P:(kc + 1) * P, hsl]
                )
                xt.append(x)
            return xt

        def proj_stile(name, xt, half, j, evict):
            st = half * (XH // QT) + j
            sl = slice(st * QT, (st + 1) * QT)
            jsl = slice(j * QT, (j + 1) * QT)
            ps = ps_qk.tile([P, QT], F32, tag="qk", name="ps")
            for kc in range(N_KCH):
                nc.tensor.matmul(
                    ps[:],
                    w_s[name][:, kc * DSL:(kc + 1) * DSL],
                    xt[kc][:, jsl],
                    start=(kc == 0),
                    stop=(kc == N_KCH - 1),
                )
            evict(ps, sl, st)

        def evict_k(ps, sl, st):
            nc.vector.tensor_scalar_add(KTs[:, sl], ps[:], b_s["k"][:])

        def evict_q(ps, sl, st):
            nc.vector.tensor_scalar_add(QTs[:, sl], ps[:], b_s["q"][:])

        def evict_v(ps, sl, st):
            nc.vector.tensor_scalar_add(VTs[:, sl], ps[:], b_s["v"][:])
            # transpose this s-tile of V^T into V natural (4 k-chunks),
            # applying the multiplicative key mask
            for h in range(HPC):
                hs = slice(h * DH, (h + 1) * DH)
                for jj in range(QT // KC):
                    kc = st * (QT // KC) + jj
                    pt = ps_qk.tile([P, DH], F32, tag="qk", name="pt")
                    nc.tensor.transpose(
                        pt[:], VTs[hs, kc * KC:(kc + 1) * KC], ident[hs, hs]
                    )
                    nc.vector.tensor_mul(
                        Vnat[h][:, kc, 0:DH],
                        pt[:],
                        mm_s[:, kc:kc + 1].to_broadcast((P, DH)),
                    )

        # DMA issue order per queue — sync: K h0, V h0, V h1;
        # scalar: Q h0, K h1, Q h1. The first QK needs K h0 + Q h0 (~23us),
        # V h0 gates only the up-front V projection; K h1 and V h1 land in
        # time for the deferred drains during q-tile 0.
        xk = [load_half("k", xkT, 0, [nc.sync], xkv_p)]
        xq = [load_half("q", xqT, 0, [nc.scalar], xq_p)]
        xv = [load_half("v", xvT, 0, [nc.sync], xkv_p)]
        xk.append(load_half("k", xkT, 1, [nc.scalar], xq_p))
        xv.append(load_half("v", xvT, 1, [nc.sync], xkv_p))
        xq.append(load_half("q", xqT, 1, [nc.scalar], xq_p))

        for j in range(XH // QT):
            proj_stile("k", xk[0], 0, j, evict_k)
        for j in range(XH // QT):
            proj_stile("q", xq[0], 0, j, evict_q)
        for j in range(XH // QT):
            proj_stile("v", xv[0], 0, j, evict_v)

        # Deferred half-1 work, split into ~1us items drained one per
        # attention step so the ScalarE exp stream never stalls on a long
        # PE block. Order interleaves K (QK needs chunk 16 at step 16)
        # with V (PV needs chunk 16 at step 17); Q half 1 drains in qt 1.
        def make_proj_parts(name, xt, j, evict):
            st = (XH // QT) + j
            sl = slice(st * QT, (st + 1) * QT)
            jsl = slice(j * QT, (j + 1) * QT)
            cell = {}

            def part0():
                ps = ps_qk.tile([P, QT], F32, tag="qk", name="ps")
                cell["ps"] = ps
                for kc in range(N_KCH // 2):
                    nc.tensor.matmul(
                        ps[:], w_s[name][:, kc * DSL:(kc + 1) * DSL],
                        xt[kc][:, jsl], start=(kc == 0), stop=False,
                    )

            def part1():
                ps = cell["ps"]
                for kc in range(N_KCH // 2, N_KCH):
                    nc.tensor.matmul(
                        ps[:], w_s[name][:, kc * DSL:(kc + 1) * DSL],
                        xt[kc][:, jsl], start=False, stop=(kc == N_KCH - 1),
                    )
                evict(ps, sl, st)

            return [part0, part1]

        def v_transpose_item(st, h):
            hs = slice(h * DH, (h + 1) * DH)
            for jj in range(QT // KC):
                kc = st * (QT // KC) + jj
                pt = ps_qk.tile([P, DH], F32, tag="qk", name="pt")
                nc.tensor.transpose(
                    pt[:], VTs[hs, kc * KC:(kc + 1) * KC], ident[hs, hs]
                )
                nc.vector.tensor_mul(
                    Vnat[h][:, kc, 0:DH],
                    pt[:],
                    mm_s[:, kc:kc + 1].to_broadcast((P, DH)),
                )

        def evict_v_bias(ps, sl, st):
            nc.vector.tensor_scalar_add(VTs[:, sl], ps[:], b_s["v"][:])

        deferred = []
        for j in range(XH // QT):
            for p in make_proj_parts("k", xk[1], j, evict_k):
                deferred.append((0, p))
            for p in make_proj_parts("v", xv[1], j, evict_v_bias):
                deferred.append((0, p))
            st = (XH // QT) + j
            for h in range(HPC):
                deferred.append((0, lambda st=st, h=h: v_transpose_item(st, h)))
        for j in range(XH // QT):
            for p in make_proj_parts("q", xq[1], j, evict_q):
                deferred.append((1, p))

        # ---- phase 2+3: attention, software-pipelined ----
        def outproj_block(qt, blk):
            st = qt * (QT // P) + blk // 2
            nt = blk % 2
            po = ps_qk.tile([P, QT], F32, tag="qk", name="po")
            nc.tensor.matmul(
                po[:],
                attnT[:, st * P:(st + 1) * P],
                wo_s[:, nt * QT:(nt + 1) * QT],
                start=True, stop=True,
            )
            ot = oevict.tile([P, QT], F32, tag="ot")
            nc.vector.tensor_copy(ot[:], po[:])
            nc.sync.dma_start(
                out_d[st * P:(st + 1) * P, nt * QT:(nt + 1) * QT], ot[:]
            )

        def emit_pv(pv, ets, kc):
            et = ets.pop(kc)
            for h in range(HPC):
                nc.tensor.matmul(
                    pv[h][:],
                    Vnat[h][:, kc, 0:DH + 1],
                    et[:, h * QT:(h + 1) * QT],
                    start=(kc == 0), stop=(kc == N_KC - 1),
                )

        for qt in range(N_ST):
            qsl = slice(qt * QT, (qt + 1) * QT)
            pv = [
                ps_pv.tile([DH + 1, QT], F32, tag="pv", name=f"pv{h}")
                for h in range(HPC)
            ]
            ets = {}
            for kc in range(N_KC):
                # both heads' QK^T into one 2-bank tile, adjacent emission
                # so the K=64 matmuls row-pack and run concurrently
                sc = ps_qk.tile([P, 2 * QT], F32, tag="qk", name="sc")
                for h in range(HPC):
                    hs = slice(h * DH, (h + 1) * DH)
                    nc.tensor.matmul(
                        sc[:, h * QT:(h + 1) * QT],
                        KTs[hs, kc * KC:(kc + 1) * KC],
                        QTs[hs, qsl],
                        start=True, stop=True,
                    )
                et = et_pool.tile([P, 2 * QT], BF16, tag="et")
                nc.scalar.activation(
                    et[:], sc[:],
                    mybir.ActivationFunctionType.Exp,
                    scale=0.125,
                )
                ets[kc] = et
                # deferred half-1 projection items ride in PE slack,
                # emitted right after the exp so the next step's QK isn't
                # pushed behind them; in qt 0 they wait until their DMAs
                # have had time to land
                if (qt > 0 or kc >= 5) and deferred and deferred[0][0] <= qt:
                    deferred.pop(0)[1]()
                # PV delayed one step: exp(kc-1) is long done, PE never
                # stalls on ScalarE
                if kc > 0:
                    emit_pv(pv, ets, kc - 1)
                if qt > 0 and 9 <= kc < 33 and kc % 3 == 0:
                    outproj_block(qt - 1, (kc - 9) // 3)
            emit_pv(pv, ets, N_KC - 1)
            # normalize: attnT[h, qsl] = pv[0:DH] * (1/pv[DH]).
            # Both pv reads (numerator copy on VectorE, denominator copy on
            # the boundary-idle ScalarE) happen immediately, so the pv PSUM
            # banks hand off to the next q-tile without waiting for the
            # reciprocal/broadcast/multiply chain.
            for h in range(HPC):
                hs = slice(h * DH, (h + 1) * DH)
                pnum = small.tile([DH, QT], F32, tag="pnum")
                nc.vector.tensor_copy(pnum[:], pv[h][0:DH, :])
                den = small.tile([1, QT], F32, tag="den")
                nc.scalar.copy(den[:], pv[h][DH:DH + 1, :])
                rec = small.tile([1, QT], F32, tag="rec")
                nc.vector.reciprocal_approx_fast(rec[:], den[:])
                bc = small.tile([DH, QT], F32, tag="bc")
                nc.gpsimd.partition_broadcast(bc[:], rec[:])
                nc.vector.tensor_mul(attnT[hs, qsl], pnum[:], bc[:])
        for blk in range(8):
            outproj_block(N_ST - 1, blk)

        if dbg:
            for name, t in (("qts", QTs), ("kts", KTs), ("vts", VTs),
                            ("attnT", attnT)):
                nc.sync.dma_start(dbg_d[name][:, :], t[:])
            nc.sync.dma_start(dbg_d["vn0"][:, :], Vnat[0][:])
            nc.sync.dma_start(dbg_d["vn1"][:, :], Vnat[1][:])

    nc.compile()
    _CACHE[key] = nc
    return nc


def _prep_in_maps(query, key, value, mask, Wq, bq, Wk, bk, Wv, bv, Wo, bo):
    f = np.float32
    qT = np.asarray(query, dtype=f)[0].T.astype(BF16_NP)
    kT = np.asarray(key, dtype=f)[0].T.astype(BF16_NP)
    vT = np.asarray(value, dtype=f)[0].T.astype(BF16_NP)
    mm = (np.asarray(mask)[0] != 0).astype(f)
    mm = np.ascontiguousarray(mm.reshape(N_KC, KC).T)  # [128, 32]
    WqT, WkT, WvT, WoT = (np.asarray(W, dtype=f).T.astype(BF16_NP)
                          for W in (Wq, Wk, Wv, Wo))
    in_maps = []
    for c in range(NCORES):
        cs = slice(c * DSL, (c + 1) * DSL)
        in_maps.append({
            "xqT": qT, "xkT": kT, "xvT": vT,
            "wqT": np.ascontiguousarray(WqT[:, cs]).reshape(N_KCH, P, DSL),
            "wkT": np.ascontiguousarray(WkT[:, cs]).reshape(N_KCH, P, DSL),
            "wvT": np.ascontiguousarray(WvT[:, cs]).reshape(N_KCH, P, DSL),
            "woT": np.ascontiguousarray(WoT[cs, :]),
            "bq": np.ascontiguousarray(bq[cs].astype(f, copy=False)).reshape(DSL, 1),
            "bk": np.ascontiguousarray(bk[cs].astype(f, copy=False)).reshape(DSL, 1),
            "bv": np.ascontiguousarray(bv[cs].astype(f, copy=False)).reshape(DSL, 1),
            "mmul": mm,
        })
    return in_maps


def run(inputs, trace=False, trace_kwargs=None, dbg=False):
    nc = _build_nc(dbg=dbg)
    in_maps = _prep_in_maps(**inputs)
    res = run_bass_kernel_spmd(
        nc, in_maps, core_ids=list(range(NCORES)), trace=trace,
        **(trace_kwargs or {}),
    )
    bo = np.asarray(inputs["bo"], dtype=np.float32)
    acc = np.zeros((S, D), dtype=np.float32)
    for r in res.results:
        acc += r["out"]
    out = (acc + bo[None, :]).astype(np.float32)[None]
    return out, res


def kernel(**inputs):
    out, _ = run(inputs, trace=False)
    return out
